# revision 27
# baseline (speedup 1.0000x reference)
"""Trainium2 Bass kernel for nn_Attention_81776177315877.

Separable-conv attention block (CMT/PVT style):
  x (B=8, 3136, 256) -> q/k/v = sepconv(dw3x3+BN+pw1x1, k/v stride 2)
  -> 8-head attention (d=32) -> proj.

Sharding: data-parallel over batch, core b <- batch b. No collectives.

v4 design (per core, channel-major layouts):
  - host: x transposed/padded/cast to bf16; q/v sepconvs fully folded
    (BN+dw taps into pointwise -> 18 K=128 PE matmuls per output block);
    k sepconv split: depthwise on DVE at startup (9 chained MACs, while
    the PE runs the v/q0 convs in parallel), pointwise on PE.
  - attention per (ic, hg, jt): S-wave (4 heads, tile_position row-packed),
    exp split between ACT (native, bf16 out) and DVE (Schraudolph-style
    bitcast fast-exp: i16 = a*S + b reinterpreted as bf16), O/D waves
    col-packed with PSUM accumulation over jt; S runs one jt ahead.
  - proj: token-major PE matmuls (lhsT = o_cm data), PSUM -> DVE
    bias-add -> f32 SBUF -> DMA out. No transposes, no scratch roundtrip.
  - PSUM: 6 banks S-pipeline (3 x [112,2,512]), 1 bank O-accum ring,
    1 bank d ring; q-conv chunk ic+1 and proj share the rings in the
    inter-phase gaps.
"""

import sys

sys.path.insert(0, "/opt/trn_rl_repo")

import numpy as np
import ml_dtypes

import concourse.bass as bass
import concourse.bacc as bacc
import concourse.mybir as mybir
import concourse.tile as tile
from concourse.bass_utils import run_bass_kernel_spmd
from concourse.masks import make_identity

FP = mybir.dt.float32
BF = mybir.dt.bfloat16
I16 = mybir.dt.int16
AF = mybir.ActivationFunctionType
ALU = mybir.AluOpType

C = 256
HEADS = 8
D = 32
HH = 56
N = HH * HH          # 3136 query tokens
HK = 28
NK = HK * HK         # 784 key tokens
PADW = HH + 2        # 58
EPS = 1e-5
SCALE = D ** -0.5

IC_CH = 8            # query rows per chunk -> 448 free
IC_F = IC_CH * HH    # 448
N_IC = HH // IC_CH   # 7
KC_CH = 14           # k/v output rows per chunk -> 392 free
KC_F = KC_CH * HK    # 392
N_KC = HK // KC_CH   # 2
JT = 112             # key tile (partitions) for attention
N_JT = NK // JT      # 7

# Schraudolph fast exp in bf16 bits: i16 = A_EXP * S + B_EXP, bits -> bf16
A_EXP = 184.6649652 * SCALE
B_EXP = 16250.5

_CACHED = {}


def _build_nc():
    nc = bacc.Bacc("TRN2", target_bir_lowering=False, debug=False, num_devices=8)

    xpad_d = nc.dram_tensor("x_pad", [128, 2, PADW, PADW], BF, kind="ExternalInput")
    const_d = {}
    for p in ("q", "k", "v"):
        const_d[p] = nc.dram_tensor(f"{p}_const", [C, 1], FP, kind="ExternalInput")
    # q, v: fully folded 9-tap weights (tap, cin128, cbi, cout)
    w9_d = {}
    for p in ("q", "v"):
        w9_d[p] = nc.dram_tensor(f"{p}_w9", [128, 9, 2, C], BF, kind="ExternalInput")
    w9_d["k"] = nc.dram_tensor("k_w9", [128, 9, 2, C], BF, kind="ExternalInput")
    pwT_d = nc.dram_tensor("proj_wT", [2, 128, C], BF, kind="ExternalInput")
    pb_d = nc.dram_tensor("proj_brep", [128, C], FP, kind="ExternalInput")
    out_d = nc.dram_tensor("out", [N, C], FP, kind="ExternalOutput")

    with tile.TileContext(nc) as tc:
        with (
            tc.tile_pool(name="persist", bufs=1) as pp,
            tc.tile_pool(name="ep", bufs=8) as ep,
            tc.tile_pool(name="rp", bufs=2) as rp,
            tc.tile_pool(name="op", bufs=3) as otp,
            tc.tile_pool(name="psS", bufs=3, space="PSUM") as psS,
            tc.tile_pool(name="psA", bufs=1, space="PSUM") as psA,
            tc.tile_pool(name="psB", bufs=1, space="PSUM") as psB,
        ):
            ident = pp.tile([128, 128], FP, tag="ident", name="ident")
            make_identity(nc, ident[:])
            ones32 = pp.tile([128, 32], BF, tag="ones32", name="ones32")
            nc.gpsimd.memset(ones32[:], 1.0)

            # ---- input / weight loads, dependency-ordered ----
            x_pad = pp.tile([128, 2, PADW, PADW], BF, tag="xpad", name="xpad")
            nc.sync.dma_start(x_pad[:, 0], xpad_d[:, 0])
            nc.scalar.dma_start(x_pad[:, 1], xpad_d[:, 1])
            w9 = {}
            consts = {}
            for p in ("k", "v", "q"):
                w9[p] = pp.tile([128, 9, 2, C], BF, tag=f"w9{p}", name=f"w9{p}")
            nc.sync.dma_start(w9["k"][:], w9_d["k"][:, :, :, :])
            nc.scalar.dma_start(w9["v"][:], w9_d["v"][:, :, :, :])
            nc.gpsimd.dma_start(w9["q"][:], w9_d["q"][:, :, :, :])
            for p in ("k", "v", "q"):
                consts[p] = [
                    pp.tile([128, 1], FP, tag=f"const_{p}{cb}", name=f"const_{p}{cb}")
                    for cb in range(2)
                ]
                for cb in range(2):
                    nc.gpsimd.dma_start(consts[p][cb][:], const_d[p][cb * 128:(cb + 1) * 128, :])
            pwT = [pp.tile([128, C], BF, tag=f"pwT{cb}", name=f"pwT{cb}") for cb in range(2)]
            for cb in range(2):
                nc.gpsimd.dma_start(pwT[cb][:], pwT_d[cb, :, :])
            pb_rep = pp.tile([128, C], FP, tag="pbrep", name="pbrep")
            nc.gpsimd.dma_start(pb_rep[:], pb_d[:, :])

            xp2 = x_pad[:].rearrange("p b (ho a) (wv c) -> p b ho a wv c", a=2, c=2)

            # ---- folded sepconv (q, v): 18 K=128 PSUM-accumulated matmuls ----
            def conv_chunk(p, dst_tiles, stride, ch_rows, wo, ch_idx, halves=(0, 1)):
                fsz = ch_rows * wo
                for half in halves:  # couts [0:128), [128:256)
                    cps = (psA if half == 0 else psB).tile(
                        [128, 448], FP, tag=f"ps{'AB'[half]}", name="cps"
                    )
                    nmm = 0
                    for tap in range(9):
                        dh, dw = tap // 3 - 1, tap % 3 - 1
                        r0 = 1 + stride * ch_idx * ch_rows + dh
                        c0 = 1 + dw
                        for cbi in range(2):
                            if stride == 1:
                                rhs = x_pad[:, cbi, r0:r0 + ch_rows, c0:c0 + wo]
                            else:
                                rhs = xp2[
                                    :, cbi,
                                    r0 // 2: r0 // 2 + ch_rows, r0 % 2,
                                    c0 // 2: c0 // 2 + wo, c0 % 2,
                                ]
                            nc.tensor.matmul(
                                cps[:, :fsz],
                                lhsT=(w9[p][:, tap, cbi, half * 128:(half + 1) * 128]),
                                rhs=(rhs),
                                start=(nmm == 0),
                                stop=(nmm == 17),
                            )
                            nmm += 1
                    nc.vector.tensor_scalar_add(
                        dst_tiles[half][:, ch_idx * fsz:(ch_idx + 1) * fsz],
                        cps[:, :fsz],
                        consts[p][half],
                    )

            # ---- startup: k, v convs + vT + q0 ----
            k_cm = [pp.tile([128, NK], BF, tag=f"kcm{cb}", name=f"kcm{cb}") for cb in range(2)]
            v_cm = [pp.tile([128, NK], FP, tag=f"vcm{cb}", name=f"vcm{cb}") for cb in range(2)]
            q_cm = [pp.tile([128, N], BF, tag=f"qcm{cb}", name=f"qcm{cb}") for cb in range(2)]
            o_cm = [pp.tile([128, N], BF, tag=f"ocm{cb}", name=f"ocm{cb}") for cb in range(2)]

            for ch in range(N_KC):
                conv_chunk("k", k_cm, 2, KC_CH, HK, ch)
            for ch in range(N_KC):
                conv_chunk("v", v_cm, 2, KC_CH, HK, ch)
            conv_chunk("q", q_cm, 1, IC_CH, HH, 0)
            v_tm = pp.tile([128, N_JT, 2, 128], BF, tag="vtm", name="vtm")
            for jt in range(N_JT):
                for cb in range(2):
                    tp = (psA if cb == 0 else psB).tile(
                        [128, 448], FP, tag=f"ps{'AB'[cb]}", name="tp"
                    )
                    nc.tensor.transpose(
                        tp[:JT, :128],
                        v_cm[cb][:, jt * JT:(jt + 1) * JT],
                        ident[:],
                    )
                    nc.vector.tensor_copy(v_tm[:JT, jt, cb, :], tp[:JT, :128])

            # ---- main loop ----
            n_tt = (N + 127) // 128  # 25 output token blocks
            ti_ready = 0

            def proj_block(st):
                pps = psB.tile([128, 448], FP, tag="psB", name="pj")
                for cb in range(2):
                    nc.tensor.matmul(
                        pps[:, :C],
                        lhsT=(o_cm[cb][:, st:st + 128]),
                        rhs=(pwT[cb][:, :]),
                        start=(cb == 0),
                        stop=(cb == 1),
                    )
                ot = otp.tile([128, C], FP, tag="ot", name="ot")
                nc.vector.scalar_tensor_tensor(
                    ot[:], pps[:, :C], 1.0, pb_rep[:], ALU.mult, ALU.add
                )
                nc.sync.dma_start(out_d[st:st + 128, :], ot[:])

            for ic in range(N_IC):
                for hg in range(2):
                    o_ps = psA.tile([128, 448], FP, tag="psA", name="o")
                    d_ps = psB.tile([128, 448], FP, tag="psB", name="d")

                    def s_pair(jt, p2):
                        s4p = psS.tile([112, 2, 512], FP, tag="s4", name="s4")
                        for hh in (2 * p2, 2 * p2 + 1):
                            nc.tensor.matmul(
                                s4p[:JT, hh % 2, :IC_F],
                                lhsT=(k_cm[hg][hh * 32:(hh + 1) * 32, jt * JT:(jt + 1) * JT]),
                                rhs=(q_cm[hg][hh * 32:(hh + 1) * 32, ic * IC_F:(ic + 1) * IC_F]),
                                start=True,
                                stop=True,
                                tile_position=(32 * hh, 0),
                                skip_group_check=True,
                            )
                        return s4p

                    s4s = [s_pair(0, 0), s_pair(0, 1)]
                    for jt in range(N_JT):
                        e4p = [None, None]
                        for p2 in range(2):
                            e4 = ep.tile([112, 2, 448], BF, tag="e", name="e")
                            # p2=0 -> ACT; p2=1 -> DVE, except every 4th jt
                            # both go to ACT (keeps DVE free for its other work)
                            if p2 == 0 or (jt + ic) % 4 == 3:
                                nc.scalar.activation(
                                    e4[:JT, :, :], s4s[p2][:JT, :, :IC_F], AF.Exp, scale=SCALE
                                )
                            else:
                                nc.vector.tensor_scalar(
                                    e4[:JT, :, :].bitcast(I16),
                                    s4s[p2][:JT, :, :IC_F],
                                    A_EXP,
                                    B_EXP,
                                    ALU.mult,
                                    ALU.add,
                                )
                            e4p[p2] = e4
                        if jt + 1 < N_JT:
                            s4s = [s_pair(jt + 1, 0), s_pair(jt + 1, 1)]
                        for p2 in range(2):
                            for hh in (2 * p2, 2 * p2 + 1):
                                nc.tensor.matmul(
                                    o_ps[hh * 32:(hh + 1) * 32, :],
                                    lhsT=(v_tm[:JT, jt, hg, hh * 32:(hh + 1) * 32]),
                                    rhs=(e4p[p2][:JT, hh % 2, :]),
                                    start=(jt == 0),
                                    stop=(jt == N_JT - 1),
                                    tile_position=(0, 32 * hh),
                                    skip_group_check=True,
                                )
                        for p2 in range(2):
                            for hh in (2 * p2, 2 * p2 + 1):
                                nc.tensor.matmul(
                                    d_ps[hh * 32:(hh + 1) * 32, :],
                                    lhsT=(ones32[:JT, :]),
                                    rhs=(e4p[p2][:JT, hh % 2, :]),
                                    start=(jt == 0),
                                    stop=(jt == N_JT - 1),
                                    tile_position=(0, 32 * hh),
                                    skip_group_check=True,
                                )

                    r_t = rp.tile([128, IC_F], FP, tag="r", name="r")
                    nc.vector.reciprocal_approx_fast(r_t[:], d_ps[:])
                    nc.vector.tensor_mul(
                        o_cm[hg][:, ic * IC_F:(ic + 1) * IC_F], o_ps[:], r_t[:]
                    )

                    # after hg0: next chunk's q conv (uses freed psum rings)
                    if hg == 0 and ic + 1 < N_IC:
                        conv_chunk("q", q_cm, 1, IC_CH, HH, ic + 1)

                # proj for all fully-covered 128-token blocks
                while ti_ready < n_tt and min(ti_ready * 128, N - 128) + 128 <= (ic + 1) * IC_F:
                    proj_block(min(ti_ready * 128, N - 128))
                    ti_ready += 1

    nc.compile()
    return nc


def _fold_common(inp):
    common = {}
    for p in ("q", "k", "v"):
        scale = inp[f"{p}_bn_g"] / np.sqrt(inp[f"{p}_bn_v"] + EPS)
        shift = inp[f"{p}_bn_b"] - inp[f"{p}_bn_m"] * scale
        w2 = inp[f"{p}_pw_w"] * scale[None, :]          # (cout, cin)
        const = (
            inp[f"{p}_pw_w"] @ (scale * inp[f"{p}_dw_b"] + shift) + inp[f"{p}_pw_b"]
        ).astype(np.float32)
        common[f"{p}_const"] = const.reshape(C, 1)
        if True:
            w9 = inp[f"{p}_dw_w"].reshape(C, 9)
            w9t = w2.T[None, :, :] * w9.T[:, :, None]   # (tap, cin, cout)
            common[f"{p}_w9"] = np.ascontiguousarray(
                w9t.reshape(9, 2, 128, C).transpose(2, 0, 1, 3)
            ).astype(ml_dtypes.bfloat16)
    common["proj_wT"] = np.ascontiguousarray(
        inp["proj_w"].T.reshape(2, 128, C)
    ).astype(ml_dtypes.bfloat16)
    common["proj_brep"] = np.ascontiguousarray(
        np.broadcast_to(inp["proj_b"].reshape(1, C), (128, C))
    ).astype(np.float32)
    return common


def prepare_x(xb):
    # xb: (3136, 256) f32 -> padded channel-major bf16 (2, 128, 58, 58)
    xt = xb.T.reshape(C, HH, HH)
    xp = np.zeros((C, PADW, PADW), np.float32)
    xp[:, 1:57, 1:57] = xt
    return np.ascontiguousarray(
        xp.reshape(2, 128, PADW, PADW).transpose(1, 0, 2, 3)
    ).astype(ml_dtypes.bfloat16)


def prepare_in_maps(inp):
    common = _fold_common(inp)
    x = inp["x"].astype(np.float32)
    return [dict(common, x_pad=prepare_x(x[b])) for b in range(x.shape[0])]


def kernel(**inputs):
    inp = {k: np.asarray(v) for k, v in inputs.items()}

    if "nc" not in _CACHED:
        _CACHED["nc"] = _build_nc()
    nc = _CACHED["nc"]

    in_maps = prepare_in_maps(inp)
    res = run_bass_kernel_spmd(nc, in_maps, list(range(len(in_maps))))
    out = np.stack([res.results[b]["out"] for b in range(len(in_maps))], axis=0)
    return out.astype(np.float32)
